# revision 77
# baseline (speedup 1.0000x reference)
"""Trainium2 Bass kernel for the Guided-Conv problem (restructured, bf16).

Math (per independent sample n, of NB = 4096):
  g_n, d_n : 24x24x9 patches of guidance / depth.
  c_n      = conv2d(g_n, conv_w, stride 8, VALID-from-SAME) + conv_b -> 3x3x9
  k_n[i]   = c_n[:, :, i] / max(||c_n[:, :, i]||_2, 1)    (per-channel 3x3 taps)
  gap_n    = mean(g_n, (y, x))                            -> 9
  W2_n     = (gap_n @ dense_w + dense_b).reshape(9, 9)    (i2 -> o2)
  r2_n[o]  = 1 / max(||W2_n[:, o]||_2, 1)
  out_n    = (depthwise(d_n, k_n) @ W2_n) * r2_n          -> 24x24x9

Device strategy (per core: 512 samples + 6 pad = 37 groups of 14):
  Partition layout q = n_local*9 + ch on 126 partitions; free = pixels.
  - Everything bf16 on the wire (validated: rel err ~6e-3 < 2e-2 gate);
    PSUM accumulation stays fp32; output returned bf16, host upcasts.
  - Weight gen via block-diagonal matmuls (lhsT = kron(eye14, w) built on
    host). gap sums via ACT copy+accum_out, interleaved into the previous
    supertile's conv loop; the dense/norm chain for supertile s+1 is
    emitted mid-conv(s) so its cross-engine latency hides under matmuls.
  - Main conv: per group, ALL nine tap matrices BD_t = mask (.) W2row (.)
    knorm[t] are built in TWO DVE ops (broadcast outer products); the PE
    accumulates 9 taps x 2 psum halves with self-loaded bf16 weights
    (LDWEIGHTS hides under the 288-col matmul streams at full clock).
  - r2 applied as the per-partition ACT scale on the PSUM->SBUF copy.
  DMA discipline (the hard-won part): engines round-robin over ALL
  outstanding transfers, so inputs are loaded JIT via pool recycling
  (bufs=2) -- an issue waits until the buffer two chunks back is consumed,
  keeping ~2 transfers in flight so completion tracks need. Consts ride
  in one packed buffer split into 8 partition-sliced DMAs (a contiguous
  transfer coalesces onto a single ~25GB/s engine; slices parallelize).
  All loads on the sync queue, stores on sync late; compute queues never
  issue DMAs.
"""

import numpy as np
import ml_dtypes

import concourse.bass as bass
from concourse import bacc
import concourse.mybir as mybir
from concourse.tile import TileContext
from concourse.bass_utils import run_bass_kernel_spmd

F = 9          # channels
P = 24         # patch size
PADW = 26      # padded patch width (SAME conv, pad 1)
KS = 3         # generated kernel size
NCORES = 8
NL = 14        # samples per group
Q = NL * F     # 126 used partitions
NGROUP = 37    # groups per core (36 full + 1 padded)
SPC = NGROUP * NL  # 518 sample slots per core (512 real)
PIX = P * P        # 576
PPIX = PADW * PADW  # 676
HALF = PIX // 2    # 288, pixels per PSUM chunk (<=512 fp32/bank)
SUPER = [4, 8, 12, 13]   # weight-gen supertile sizes (sum = 37)
DCH = [4, 6, 6, 7, 7, 7]  # din chunk sizes (sum = 37)
OCH = [8, 8, 8, 6, 4, 3]     # groups per output-store DMA (small tail)

F32 = mybir.dt.float32
BF16 = mybir.dt.bfloat16
NPBF = ml_dtypes.bfloat16


def build_program():
    nc = bacc.Bacc("TRN2", target_bir_lowering=False, debug=False,
                   num_devices=NCORES)

    gin = nc.dram_tensor("gin", [Q, NGROUP, PIX], BF16, kind="ExternalInput").ap()
    din = nc.dram_tensor("din", [Q, NGROUP, PPIX], BF16, kind="ExternalInput").ap()
    # all consts packed into one buffer -> ONE early DMA:
    # [0:1134) lhsA (126 rows), [1134:2268) lhsD (127), [2268:3402) lhsD2
    # (127), [3402:3528) mask (126), [3528:3530) conv_b as fp32 bytes.
    # Row stride padded to 4096 so the DRAM read is NON-contiguous -- a
    # fully contiguous transfer coalesces onto a single DMA engine.
    cpk = nc.dram_tensor("cpk", [Q + 1, 3530], BF16, kind="ExternalInput").ap()
    outd = nc.dram_tensor("out", [Q, NGROUP, PIX], BF16, kind="ExternalOutput").ap()

    supers = []
    g0 = 0
    for ng in SUPER:
        supers.append((g0, ng))
        g0 += ng
    dchunks = []
    c0 = 0
    for cn in DCH:
        dchunks.append((c0, cn))
        c0 += cn

    with TileContext(nc) as tc:
        with (
            tc.tile_pool(name="consts", bufs=1) as cpool,
            tc.tile_pool(name="gpool", bufs=3) as gpool,
            tc.tile_pool(name="dpool", bufs=2) as dpool,
            tc.tile_pool(name="opool", bufs=3) as opool,
            tc.tile_pool(name="small", bufs=1) as spool,
            tc.tile_pool(name="gapp", bufs=2) as gappool,
            tc.tile_pool(name="bd", bufs=14) as bdpool,
            tc.tile_pool(name="ps_c", bufs=1, space="PSUM") as pcpool,
            tc.tile_pool(name="ps_d", bufs=1, space="PSUM") as pdpool,
            tc.tile_pool(name="ps_main", bufs=5, space="PSUM") as pmpool,
        ):
            # ---- all input DMAs hoisted to program start, on idle queues
            # (issue cost is ~126 descriptors each; keep off the ACT/DVE
            # compute queues and use few, large transfers -- each transfer
            # is spread over all ~14 DMA engines by the packetizer).
            # Strict need-order on ONE ring: transfers drain FIFO per ring
            # round-robined over the shared DMA engines, so bulk loads
            # issued later cannot starve latency-critical small ones.
            # 8 partition-sliced loads: issue order = packet order in the
            # engine FIFOs, and each issue round-robins to a different
            # engine, so slices transfer in parallel ahead of the bulk.
            csb = cpool.tile([Q + 1, 3530], BF16, tag="cpk")
            for p0 in range(0, Q + 1, 16):
                p1 = min(p0 + 16, Q + 1)
                nc.sync.dma_start(out=csb[p0:p1, :], in_=cpk[p0:p1, :])
            lhsA_sb = csb[0:Q, 0:1134].rearrange("p (t q) -> p t q", q=Q)
            lhsD_sb = csb[:, 1134:2268].rearrange("p (t q) -> p t q", q=Q)
            lhsD2_sb = csb[:, 2268:3402].rearrange("p (t q) -> p t q", q=Q)
            mask_sb = csb[0:Q, 3402:3528].rearrange("p (a b) -> p a b", b=F)
            # JIT-staged loading: DMA engines round-robin over ALL
            # outstanding transfers, so hoisting everything means nothing
            # lands until the aggregate drain (~50us). Instead, pool
            # recycling (bufs=2) makes each load's issue WAIT until the
            # buffer two chunks back is consumed -- only ~2 transfers
            # outstanding, completion tracks need.
            GSBW = max(ng for _, ng in supers) * PIX
            DSBW = max(cn for _, cn in dchunks) * PPIX
            gtiles = {}
            dtiles = {}

            def load_g(si):
                s0, ng = supers[si]
                t = gpool.tile([Q, GSBW], BF16, tag="gsb")
                gtiles[si] = t
                nc.sync.dma_start(
                    out=t[:, :ng * PIX],
                    in_=gin[:, s0:s0 + ng].rearrange("p g f -> p (g f)"))

            def load_d(ci):
                c0, cn = dchunks[ci]
                t = dpool.tile([Q, DSBW], BF16, tag="dsb")
                dtiles[ci] = t
                nc.sync.dma_start(
                    out=t[:, :cn * PPIX],
                    in_=din[:, c0:c0 + cn].rearrange("p g f -> p (g f)"))

            convb_sb = csb[0:Q, 3528:3530].bitcast(F32)
            load_g(0)
            load_d(0)
            load_g(1)
            load_d(1)

            # ---- persistent per-core small tensors ----
            craw = spool.tile([Q, NGROUP, F], F32, tag="craw")     # c + conv_b
            knorm = spool.tile([Q, NGROUP, F], F32, tag="knorm")   # normalized taps
            w2 = spool.tile([Q, NGROUP, F], BF16, tag="w2")        # raw W2 (D2 layout)
            r2 = spool.tile([Q, NGROUP], F32, tag="r2")            # 1/max(n2,1)
            r1 = spool.tile([Q, NGROUP], F32, tag="r1")            # 1/max(n1,1)
            sq = spool.tile([Q, NGROUP, F], F32, tag="sq")         # scratch squares
            s1 = spool.tile([Q, NGROUP], F32, tag="s1")            # scratch sums
            gap32 = spool.tile([Q, NGROUP], F32, tag="gap32")      # fp32 pixel sums

            def emit_gap(g):
                # per-group pixel SUM, alternating between the ACT queue
                # (copy + accum_out) and DVE (tensor_reduce) to balance
                # engine load; emitted interleaved with the previous
                # supertile's conv to avoid serial bursts.
                si = next(i for i, (s0, ng) in enumerate(supers)
                          if s0 <= g < s0 + ng)
                s0 = supers[si][0]
                gv = gtiles[si][:, (g - s0) * PIX:(g - s0 + 1) * PIX]
                if si == 0 and g % 2 == 0:
                    # prologue: split s0's sums across DVE and ACT so the
                    # two chains run in parallel (both engines are idle,
                    # and the first dense waits on ALL of them).
                    nc.vector.tensor_reduce(
                        out=gap32[:, g:g + 1],
                        in_=gv.rearrange("p (o f) -> p o f", o=1),
                        axis=mybir.AxisListType.X, op=mybir.AluOpType.add)
                else:
                    scr = gappool.tile([Q, PIX], BF16, tag="scr")
                    nc.scalar.activation(
                        out=scr, in_=gv,
                        func=mybir.ActivationFunctionType.Copy,
                        bias=0.0, scale=1.0,
                        accum_out=gap32[:, g:g + 1])

            def emit_wgenA(si):
                # conv A (strided conv -> craw): only needs lhsA + gsb, so
                # it can run early and keep the PE warm during the prologue.
                s0, ng = supers[si]
                gsl = slice(s0, s0 + ng)
                gsb = gtiles[si][:, :ng * PIX]
                psc = pcpool.tile([Q, ng, F], F32, tag="psc")
                gwin = gsb.rearrange(
                    "p (g oy yr ox xr) -> p g oy ox yr xr",
                    g=ng, oy=KS, yr=8, ox=KS, xr=8)
                for t in range(KS * KS):
                    ky, kx = divmod(t, KS)
                    nc.tensor.matmul(
                        psc,
                        lhsT=lhsA_sb[:, t, :],
                        rhs=gwin[:, :, :, :, ky, kx],
                        start=(t == 0), stop=(t == KS * KS - 1),
                        skip_group_check=True)

                # craw = psc + conv_b (per-partition bias)
                nc.scalar.activation(
                    out=craw[:, gsl, :], in_=psc,
                    func=mybir.ActivationFunctionType.Identity,
                    bias=convb_sb, scale=1.0)

            def emit_wgenB(si):
                s0, ng = supers[si]
                gsl = slice(s0, s0 + ng)
                # bf16 gap copy for the dense rhs; row 126 reads 1.0 so the
                # K=127 matmul adds dense_b.
                gap = gappool.tile([128, ng], BF16, tag="gap")
                nc.vector.memset(gap, 1.0)
                nc.scalar.copy(out=gap[0:Q, :], in_=gap32[:, gsl])

                # dense layer, both layouts (D for the norm, D2 for values)
                psD = pdpool.tile([Q, F, ng], F32, tag="psD")
                psD2 = pdpool.tile([Q, F, ng], F32, tag="psD2")
                for j in range(F):
                    nc.tensor.matmul(psD[:, j, :], lhsT=lhsD_sb[:, j, :],
                                     rhs=gap[0:Q + 1, :],
                                     start=True, stop=True,
                                     skip_group_check=True)
                for j in range(F):
                    nc.tensor.matmul(psD2[:, j, :], lhsT=lhsD2_sb[:, j, :],
                                     rhs=gap[0:Q + 1, :],
                                     start=True, stop=True,
                                     skip_group_check=True)

                # r2 = 1/max(||W2[:,o]||, 1):  sum_i2 D^2 per (n,o2)
                nc.scalar.square(out=sq[:, gsl, :],
                                 in_=psD.rearrange("p i g -> p g i"))
                nc.vector.tensor_reduce(
                    out=s1[:, gsl], in_=sq[:, gsl, :],
                    axis=mybir.AxisListType.X, op=mybir.AluOpType.add)
                nc.scalar.sqrt(out=s1[:, gsl], in_=s1[:, gsl])
                nc.vector.tensor_scalar_max(r2[:, gsl], s1[:, gsl], 1.0)
                nc.vector.reciprocal(r2[:, gsl], r2[:, gsl])

                # W2 raw values, group-major, stored bf16 for the BDW build
                nc.scalar.copy(out=w2[:, gsl, :],
                               in_=psD2.rearrange("p o g -> p g o"))

                # r1 = 1/max(||c||, 1) per (n, ch); knorm = craw * r1
                nc.scalar.square(out=sq[:, gsl, :], in_=craw[:, gsl, :])
                nc.vector.tensor_reduce(
                    out=s1[:, gsl], in_=sq[:, gsl, :],
                    axis=mybir.AxisListType.X, op=mybir.AluOpType.add)
                nc.scalar.sqrt(out=s1[:, gsl], in_=s1[:, gsl])
                nc.vector.tensor_scalar_max(r1[:, gsl], s1[:, gsl], 1.0)
                nc.vector.reciprocal(r1[:, gsl], r1[:, gsl])
                nc.vector.tensor_mul(
                    out=knorm[:, gsl, :], in0=craw[:, gsl, :],
                    in1=r1[:, gsl].unsqueeze(2).broadcast_to([Q, ng, F]))

            # prologue: conv-A for the first three supertiles keeps the PE
            # busy (and ramping to full clock) while the s0 dense/norm
            # chain's cross-engine latency plays out.
            EARLY_A = min(2, len(supers))
            emit_wgenA(0)
            for g in range(supers[0][0], supers[0][0] + supers[0][1]):
                emit_gap(g)
            for si in range(1, EARLY_A):
                emit_wgenA(si)
            emit_wgenB(0)

            oends = set()
            acc = 0
            for on_ in OCH:
                acc += on_
                oends.add(acc)
            osb = None
            ochunk_start = 0
            for si, (s0, ng) in enumerate(supers):
                # interleave: next supertile's gap sums into this conv loop,
                # and emit its whole weight-gen mid-conv so the dense/norm
                # chain latency hides under this supertile's matmuls.
                if si + 1 < len(supers):
                    nxt0, nxtn = supers[si + 1]
                    pending = list(range(nxt0, nxt0 + nxtn))
                else:
                    pending = []
                wgen_at = max(ng * 3 // 5, 1) if pending else ng + 1
                per_group = -(-len(pending) // wgen_at) if pending else 0
                if si + 1 < len(supers) and si + 1 not in gtiles:
                    load_g(si + 1)

                for g in range(s0, s0 + ng):
                    ci = next(i for i, (c0, cn) in enumerate(dchunks)
                              if c0 <= g < c0 + cn)
                    c0 = dchunks[ci][0]
                    # JIT prefetch: entering chunk ci -> issue chunk ci+2
                    if g == c0 and ci + 2 < len(dchunks):
                        load_d(ci + 2)
                    drows = dtiles[ci][:, (g - c0) * PPIX:(g - c0 + 1) * PPIX
                                       ].rearrange("p (r c) -> p r c", c=PADW)
                    if osb is None:
                        ochunk_start = g
                        osb = opool.tile([Q, max(OCH) * PIX], BF16,
                                         tag="osb")

                    # BDW = mask (.) broadcast(W2 row), tap-independent
                    bdw = bdpool.tile([Q, NL * F], BF16, tag="bdw")
                    nc.vector.tensor_mul(
                        out=bdw.rearrange("p (a b) -> p a b", b=F),
                        in0=w2[:, g, :].unsqueeze(1).broadcast_to([Q, NL, F]),
                        in1=mask_sb)

                    # ALL nine per-tap BD_t = knorm[t] * BDW in one DVE op
                    # (outer product via dual broadcast); the per-op fixed
                    # overhead of 9 small tensor_scalars dominated DVE.
                    # EXCEPT at each supertile's first group, where the
                    # 1.3us monolithic build sits on the critical path
                    # right after the norm chain: build per-tap there so
                    # the PE starts after the first ~170ns tensor_scalar.
                    bdall = bdpool.tile([Q, KS * KS, NL * F], BF16,
                                        tag="bdall")
                    if g != s0:
                        nc.vector.tensor_mul(
                            out=bdall,
                            in0=bdw.unsqueeze(1).broadcast_to(
                                [Q, KS * KS, NL * F]),
                            in1=knorm[:, g, :].unsqueeze(2).broadcast_to(
                                [Q, KS * KS, NL * F]))

                    pm0 = pmpool.tile([Q, HALF], F32, tag="pm")
                    pm1 = pmpool.tile([Q, HALF], F32, tag="pm")
                    pms = [pm0, pm1]
                    for t in range(KS * KS):
                        ky, kx = divmod(t, KS)
                        if g == s0:
                            nc.vector.tensor_scalar_mul(
                                bdall[:, t, :], bdw, knorm[:, g, t:t + 1])
                        for h in range(2):
                            rhs = drows[:, h * 12 + ky:h * 12 + ky + 12,
                                        kx:kx + P]
                            nc.tensor.matmul(
                                pms[h], lhsT=bdall[:, t, :], rhs=rhs,
                                start=(t == 0), stop=(t == KS * KS - 1),
                                skip_group_check=True)

                    gl = g - ochunk_start
                    for h in range(2):
                        nc.scalar.activation(
                            out=osb[:, gl * PIX + h * HALF:
                                    gl * PIX + (h + 1) * HALF],
                            in_=pms[h],
                            func=mybir.ActivationFunctionType.Copy,
                            bias=0.0, scale=r2[:, g:g + 1])

                    for _ in range(per_group):
                        if pending:
                            emit_gap(pending.pop(0))
                    if g - s0 + 1 == wgen_at and si + 1 < len(supers):
                        while pending:
                            emit_gap(pending.pop(0))
                        if si + 1 >= EARLY_A:
                            emit_wgenA(si + 1)
                        emit_wgenB(si + 1)

                    if g + 1 in oends:
                        on = g - ochunk_start + 1
                        nc.sync.dma_start(
                            out=outd[:, ochunk_start:ochunk_start + on
                                     ].rearrange("p g f -> p (g f)"),
                            in_=osb[:, :on * PIX])
                        osb = None

    nc.compile()
    return nc


def _host_prep(guidance, depth, conv_w, conv_b, dense_w, dense_b):
    B, H, W, _ = guidance.shape
    nh, nw = H // P, W // P
    NB = B * nh * nw

    def to_samples(x):
        # (B,H,W,F) -> (NB, P, P, F), sample order = flat (b, i, j)
        return (x.reshape(B, nh, P, nw, P, F)
                 .transpose(0, 1, 3, 2, 4, 5)
                 .reshape(NB, P, P, F))

    gs = to_samples(np.ascontiguousarray(guidance))
    ds = to_samples(np.ascontiguousarray(depth))

    in_maps = []
    for c in range(NCORES):
        gsl = gs[c * 512:(c + 1) * 512]
        dsl = ds[c * 512:(c + 1) * 512]
        gpad = np.zeros((SPC, P, P, F), np.float32)
        gpad[:512] = gsl
        dpad = np.zeros((SPC, PADW, PADW, F), np.float32)
        dpad[:512, 1:P + 1, 1:P + 1] = dsl
        # (SPC, y, x, ch) -> [NGROUP, 126, pix]  with q = n_local*9 + ch
        gq = (gpad.reshape(NGROUP, NL, P, P, F)
                  .transpose(1, 4, 0, 2, 3)
                  .reshape(Q, NGROUP, PIX))
        dq = (dpad.reshape(NGROUP, NL, PADW, PADW, F)
                  .transpose(1, 4, 0, 2, 3)
                  .reshape(Q, NGROUP, PPIX))
        in_maps.append({"gin": np.ascontiguousarray(gq).astype(NPBF),
                        "din": np.ascontiguousarray(dq).astype(NPBF)})

    eye = np.eye(NL, dtype=np.float32)
    lhsA = np.zeros((KS * KS, Q, Q), np.float32)
    for t in range(KS * KS):
        ky, kx = divmod(t, KS)
        lhsA[t] = np.kron(eye, conv_w[ky, kx])
    lhsA = np.ascontiguousarray(lhsA.transpose(1, 0, 2))      # [Q, 9, Q]
    lhsD = np.zeros((F, Q + 1, Q), np.float32)
    lhsD2 = np.zeros((F, Q + 1, Q), np.float32)
    dws = dense_w.astype(np.float32) / PIX  # gap arrives as a SUM over pixels
    for j in range(F):
        lhsD[j, :Q] = np.kron(eye, dws[:, j * F:(j + 1) * F])
        lhsD[j, Q] = np.tile(dense_b[j * F:(j + 1) * F], NL)
        lhsD2[j, :Q] = np.kron(eye, dws[:, j::F])
        lhsD2[j, Q] = np.tile(dense_b[j::F], NL)
    lhsD = np.ascontiguousarray(lhsD.transpose(1, 0, 2))      # [Q+1, 9, Q]
    lhsD2 = np.ascontiguousarray(lhsD2.transpose(1, 0, 2))
    mask = np.kron(eye, np.ones((F, F), np.float32))
    convb = np.tile(conv_b.astype(np.float32), NL)[:, None]

    cpk = np.zeros((Q + 1, 3530), NPBF)
    cpk[0:Q, 0:1134] = lhsA.astype(NPBF).reshape(Q, 1134)
    cpk[:, 1134:2268] = lhsD.astype(NPBF).reshape(Q + 1, 1134)
    cpk[:, 2268:3402] = lhsD2.astype(NPBF).reshape(Q + 1, 1134)
    cpk[0:Q, 3402:3528] = mask.astype(NPBF)
    cpk[0:Q, 3528:3530] = (np.ascontiguousarray(convb)
                           .view(np.uint16).view(NPBF))
    consts = {"cpk": cpk}
    for m in in_maps:
        m.update(consts)
    return in_maps


_CACHED_NC = None


def run(inputs, trace=False, **kw):
    """Build (cached), run on 8 cores, return (full_output, BassKernelResults)."""
    global _CACHED_NC
    inputs = {k: np.asarray(v, np.float32) for k, v in inputs.items()}
    in_maps = _host_prep(**inputs)
    if _CACHED_NC is None:
        _CACHED_NC = build_program()
    res = run_bass_kernel_spmd(_CACHED_NC, in_maps, list(range(NCORES)),
                               trace=trace, **kw)
    outs = []
    for c in range(NCORES):
        o = np.asarray(res.results[c]["out"]).astype(np.float32)
        o = o.reshape(NL, F, NGROUP, P, P)
        o = o.transpose(2, 0, 3, 4, 1).reshape(SPC, P, P, F)[:512]
        outs.append(o)
    full = np.concatenate(outs, 0)  # (4096, 24, 24, 9) in (b, i, j) order
    B, H, W = 16, 384, 384
    return full.reshape(B, H, W, F), res


def kernel(**inputs):
    out, _ = run(inputs, trace=False)
    return out


# revision 78
# speedup vs baseline: 1.0149x; 1.0149x over previous
"""Trainium2 Bass kernel for the Guided-Conv problem (restructured, bf16).

Math (per independent sample n, of NB = 4096):
  g_n, d_n : 24x24x9 patches of guidance / depth.
  c_n      = conv2d(g_n, conv_w, stride 8, VALID-from-SAME) + conv_b -> 3x3x9
  k_n[i]   = c_n[:, :, i] / max(||c_n[:, :, i]||_2, 1)    (per-channel 3x3 taps)
  gap_n    = mean(g_n, (y, x))                            -> 9
  W2_n     = (gap_n @ dense_w + dense_b).reshape(9, 9)    (i2 -> o2)
  r2_n[o]  = 1 / max(||W2_n[:, o]||_2, 1)
  out_n    = (depthwise(d_n, k_n) @ W2_n) * r2_n          -> 24x24x9

Device strategy (per core: 512 samples + 6 pad = 37 groups of 14):
  Partition layout q = n_local*9 + ch on 126 partitions; free = pixels.
  - Everything bf16 on the wire (validated: rel err ~6e-3 < 2e-2 gate);
    PSUM accumulation stays fp32; output returned bf16, host upcasts.
  - Weight gen via block-diagonal matmuls (lhsT = kron(eye14, w) built on
    host). gap sums via ACT copy+accum_out, interleaved into the previous
    supertile's conv loop; the dense/norm chain for supertile s+1 is
    emitted mid-conv(s) so its cross-engine latency hides under matmuls.
  - Main conv: per group, ALL nine tap matrices BD_t = mask (.) W2row (.)
    knorm[t] are built in TWO DVE ops (broadcast outer products); the PE
    accumulates 9 taps x 2 psum halves with self-loaded bf16 weights
    (LDWEIGHTS hides under the 288-col matmul streams at full clock).
  - r2 applied as the per-partition ACT scale on the PSUM->SBUF copy.
  DMA discipline (the hard-won part): engines round-robin over ALL
  outstanding transfers, so inputs are loaded JIT via pool recycling
  (bufs=2) -- an issue waits until the buffer two chunks back is consumed,
  keeping ~2 transfers in flight so completion tracks need. Consts ride
  in one packed buffer split into 8 partition-sliced DMAs (a contiguous
  transfer coalesces onto a single ~25GB/s engine; slices parallelize).
  All loads on the sync queue, stores on sync late; compute queues never
  issue DMAs.
"""

import numpy as np
import ml_dtypes

import concourse.bass as bass
from concourse import bacc
import concourse.mybir as mybir
from concourse.tile import TileContext
from concourse.bass_utils import run_bass_kernel_spmd

F = 9          # channels
P = 24         # patch size
PADW = 26      # padded patch width (SAME conv, pad 1)
KS = 3         # generated kernel size
NCORES = 8
NL = 14        # samples per group
Q = NL * F     # 126 used partitions
NGROUP = 37    # groups per core (36 full + 1 padded)
SPC = NGROUP * NL  # 518 sample slots per core (512 real)
PIX = P * P        # 576
PPIX = PADW * PADW  # 676
HALF = PIX // 2    # 288, pixels per PSUM chunk (<=512 fp32/bank)
SUPER = [4, 8, 12, 13]   # weight-gen supertile sizes (sum = 37)
DCH = [4, 6, 6, 7, 7, 7]  # din chunk sizes (sum = 37)
OCH = [8, 8, 8, 6, 4, 3]     # groups per output-store DMA (small tail)

F32 = mybir.dt.float32
BF16 = mybir.dt.bfloat16
NPBF = ml_dtypes.bfloat16


def build_program():
    nc = bacc.Bacc("TRN2", target_bir_lowering=False, debug=False,
                   num_devices=NCORES)

    gin = nc.dram_tensor("gin", [Q, NGROUP, PIX], BF16, kind="ExternalInput").ap()
    din = nc.dram_tensor("din", [Q, NGROUP, PPIX], BF16, kind="ExternalInput").ap()
    # all consts packed into one buffer -> ONE early DMA:
    # [0:1134) lhsA (126 rows), [1134:2268) lhsD (127), [2268:3402) lhsD2
    # (127), [3402:3528) mask (126), [3528:3530) conv_b as fp32 bytes.
    # Row stride padded to 4096 so the DRAM read is NON-contiguous -- a
    # fully contiguous transfer coalesces onto a single DMA engine.
    cpk = nc.dram_tensor("cpk", [Q + 1, 3530], BF16, kind="ExternalInput").ap()
    outd = nc.dram_tensor("out", [Q, NGROUP, PIX], BF16, kind="ExternalOutput").ap()

    supers = []
    g0 = 0
    for ng in SUPER:
        supers.append((g0, ng))
        g0 += ng
    dchunks = []
    c0 = 0
    for cn in DCH:
        dchunks.append((c0, cn))
        c0 += cn

    with TileContext(nc) as tc:
        with (
            tc.tile_pool(name="consts", bufs=1) as cpool,
            tc.tile_pool(name="gpool", bufs=3) as gpool,
            tc.tile_pool(name="dpool", bufs=2) as dpool,
            tc.tile_pool(name="opool", bufs=3) as opool,
            tc.tile_pool(name="small", bufs=1) as spool,
            tc.tile_pool(name="gapp", bufs=2) as gappool,
            tc.tile_pool(name="bd", bufs=14) as bdpool,
            tc.tile_pool(name="ps_c", bufs=1, space="PSUM") as pcpool,
            tc.tile_pool(name="ps_d", bufs=1, space="PSUM") as pdpool,
            tc.tile_pool(name="ps_main", bufs=5, space="PSUM") as pmpool,
        ):
            # ---- all input DMAs hoisted to program start, on idle queues
            # (issue cost is ~126 descriptors each; keep off the ACT/DVE
            # compute queues and use few, large transfers -- each transfer
            # is spread over all ~14 DMA engines by the packetizer).
            # Strict need-order on ONE ring: transfers drain FIFO per ring
            # round-robined over the shared DMA engines, so bulk loads
            # issued later cannot starve latency-critical small ones.
            # 8 partition-sliced loads: issue order = packet order in the
            # engine FIFOs, and each issue round-robins to a different
            # engine, so slices transfer in parallel ahead of the bulk.
            csb = cpool.tile([Q + 1, 3530], BF16, tag="cpk")
            for p0 in range(0, Q + 1, 16):
                p1 = min(p0 + 16, Q + 1)
                nc.sync.dma_start(out=csb[p0:p1, :], in_=cpk[p0:p1, :])
            lhsA_sb = csb[0:Q, 0:1134].rearrange("p (t q) -> p t q", q=Q)
            lhsD_sb = csb[:, 1134:2268].rearrange("p (t q) -> p t q", q=Q)
            lhsD2_sb = csb[:, 2268:3402].rearrange("p (t q) -> p t q", q=Q)
            mask_sb = csb[0:Q, 3402:3528].rearrange("p (a b) -> p a b", b=F)
            # JIT-staged loading: DMA engines round-robin over ALL
            # outstanding transfers, so hoisting everything means nothing
            # lands until the aggregate drain (~50us). Instead, pool
            # recycling (bufs=2) makes each load's issue WAIT until the
            # buffer two chunks back is consumed -- only ~2 transfers
            # outstanding, completion tracks need.
            GSBW = max(ng for _, ng in supers) * PIX
            DSBW = max(cn for _, cn in dchunks) * PPIX
            gtiles = {}
            dtiles = {}

            def load_g(si):
                s0, ng = supers[si]
                t = gpool.tile([Q, GSBW], BF16, tag="gsb")
                gtiles[si] = t
                nc.sync.dma_start(
                    out=t[:, :ng * PIX],
                    in_=gin[:, s0:s0 + ng].rearrange("p g f -> p (g f)"))

            def load_d(ci):
                c0, cn = dchunks[ci]
                t = dpool.tile([Q, DSBW], BF16, tag="dsb")
                dtiles[ci] = t
                nc.sync.dma_start(
                    out=t[:, :cn * PPIX],
                    in_=din[:, c0:c0 + cn].rearrange("p g f -> p (g f)"))

            convb_sb = csb[0:Q, 3528:3530].bitcast(F32)
            load_g(0)
            load_d(0)
            load_g(1)
            load_d(1)

            # ---- persistent per-core small tensors ----
            craw = spool.tile([Q, NGROUP, F], F32, tag="craw")     # c + conv_b
            knorm = spool.tile([Q, NGROUP, F], F32, tag="knorm")   # normalized taps
            w2 = spool.tile([Q, NGROUP, F], BF16, tag="w2")        # raw W2 (D2 layout)
            r2 = spool.tile([Q, NGROUP], F32, tag="r2")            # 1/max(n2,1)
            r1 = spool.tile([Q, NGROUP], F32, tag="r1")            # 1/max(n1,1)
            sq = spool.tile([Q, NGROUP, F], F32, tag="sq")         # scratch squares
            s1 = spool.tile([Q, NGROUP], F32, tag="s1")            # scratch sums
            gap32 = spool.tile([Q, NGROUP], F32, tag="gap32")      # fp32 pixel sums

            def emit_gap(g):
                # per-group pixel SUM, alternating between the ACT queue
                # (copy + accum_out) and DVE (tensor_reduce) to balance
                # engine load; emitted interleaved with the previous
                # supertile's conv to avoid serial bursts.
                si = next(i for i, (s0, ng) in enumerate(supers)
                          if s0 <= g < s0 + ng)
                s0 = supers[si][0]
                gv = gtiles[si][:, (g - s0) * PIX:(g - s0 + 1) * PIX]
                if si == 0 and g % 2 == 0:
                    # prologue: split s0's sums across DVE and ACT so the
                    # two chains run in parallel (both engines are idle,
                    # and the first dense waits on ALL of them).
                    nc.vector.tensor_reduce(
                        out=gap32[:, g:g + 1],
                        in_=gv.rearrange("p (o f) -> p o f", o=1),
                        axis=mybir.AxisListType.X, op=mybir.AluOpType.add)
                else:
                    scr = gappool.tile([Q, PIX], BF16, tag="scr")
                    nc.scalar.activation(
                        out=scr, in_=gv,
                        func=mybir.ActivationFunctionType.Copy,
                        bias=0.0, scale=1.0,
                        accum_out=gap32[:, g:g + 1])

            def emit_wgenA(si):
                # conv A (strided conv -> craw): only needs lhsA + gsb, so
                # it can run early and keep the PE warm during the prologue.
                s0, ng = supers[si]
                gsl = slice(s0, s0 + ng)
                gsb = gtiles[si][:, :ng * PIX]
                psc = pcpool.tile([Q, ng, F], F32, tag="psc")
                gwin = gsb.rearrange(
                    "p (g oy yr ox xr) -> p g oy ox yr xr",
                    g=ng, oy=KS, yr=8, ox=KS, xr=8)
                for t in range(KS * KS):
                    ky, kx = divmod(t, KS)
                    nc.tensor.matmul(
                        psc,
                        lhsT=lhsA_sb[:, t, :],
                        rhs=gwin[:, :, :, :, ky, kx],
                        start=(t == 0), stop=(t == KS * KS - 1),
                        skip_group_check=True)

                # craw = psc + conv_b (per-partition bias)
                nc.scalar.activation(
                    out=craw[:, gsl, :], in_=psc,
                    func=mybir.ActivationFunctionType.Identity,
                    bias=convb_sb, scale=1.0)

            def emit_wgenB(si):
                s0, ng = supers[si]
                gsl = slice(s0, s0 + ng)
                # bf16 gap copy for the dense rhs; row 126 reads 1.0 so the
                # K=127 matmul adds dense_b.
                gap = gappool.tile([128, ng], BF16, tag="gap")
                nc.vector.memset(gap, 1.0)
                nc.scalar.copy(out=gap[0:Q, :], in_=gap32[:, gsl])

                # dense layer, both layouts (D for the norm, D2 for values)
                psD = pdpool.tile([Q, F, ng], F32, tag="psD")
                psD2 = pdpool.tile([Q, F, ng], F32, tag="psD2")
                for j in range(F):
                    nc.tensor.matmul(psD[:, j, :], lhsT=lhsD_sb[:, j, :],
                                     rhs=gap[0:Q + 1, :],
                                     start=True, stop=True,
                                     skip_group_check=True)
                for j in range(F):
                    nc.tensor.matmul(psD2[:, j, :], lhsT=lhsD2_sb[:, j, :],
                                     rhs=gap[0:Q + 1, :],
                                     start=True, stop=True,
                                     skip_group_check=True)

                # r2 = 1/max(||W2[:,o]||, 1):  sum_i2 D^2 per (n,o2)
                nc.scalar.square(out=sq[:, gsl, :],
                                 in_=psD.rearrange("p i g -> p g i"))
                nc.vector.tensor_reduce(
                    out=s1[:, gsl], in_=sq[:, gsl, :],
                    axis=mybir.AxisListType.X, op=mybir.AluOpType.add)
                nc.scalar.sqrt(out=s1[:, gsl], in_=s1[:, gsl])
                nc.vector.tensor_scalar_max(r2[:, gsl], s1[:, gsl], 1.0)
                nc.vector.reciprocal(r2[:, gsl], r2[:, gsl])

                # W2 raw values, group-major, stored bf16 for the BDW build
                nc.scalar.copy(out=w2[:, gsl, :],
                               in_=psD2.rearrange("p o g -> p g o"))

                # r1 = 1/max(||c||, 1) per (n, ch); knorm = craw * r1
                nc.scalar.square(out=sq[:, gsl, :], in_=craw[:, gsl, :])
                nc.vector.tensor_reduce(
                    out=s1[:, gsl], in_=sq[:, gsl, :],
                    axis=mybir.AxisListType.X, op=mybir.AluOpType.add)
                nc.scalar.sqrt(out=s1[:, gsl], in_=s1[:, gsl])
                nc.vector.tensor_scalar_max(r1[:, gsl], s1[:, gsl], 1.0)
                nc.vector.reciprocal(r1[:, gsl], r1[:, gsl])
                nc.vector.tensor_mul(
                    out=knorm[:, gsl, :], in0=craw[:, gsl, :],
                    in1=r1[:, gsl].unsqueeze(2).broadcast_to([Q, ng, F]))

            # prologue: conv-A for the first three supertiles keeps the PE
            # busy (and ramping to full clock) while the s0 dense/norm
            # chain's cross-engine latency plays out.
            EARLY_A = min(2, len(supers))
            emit_wgenA(0)
            for g in range(supers[0][0], supers[0][0] + supers[0][1]):
                emit_gap(g)
            for si in range(1, EARLY_A):
                emit_wgenA(si)
            emit_wgenB(0)

            oends = set()
            acc = 0
            for on_ in OCH:
                acc += on_
                oends.add(acc)
            osb = None
            ochunk_start = 0
            for si, (s0, ng) in enumerate(supers):
                # interleave: next supertile's gap sums into this conv loop,
                # and emit its whole weight-gen mid-conv so the dense/norm
                # chain latency hides under this supertile's matmuls.
                if si + 1 < len(supers):
                    nxt0, nxtn = supers[si + 1]
                    pending = list(range(nxt0, nxt0 + nxtn))
                else:
                    pending = []
                wgen_at = max(ng * 3 // 5, 1) if pending else ng + 1
                per_group = -(-len(pending) // wgen_at) if pending else 0
                if si + 1 < len(supers) and si + 1 not in gtiles:
                    load_g(si + 1)

                for g in range(s0, s0 + ng):
                    ci = next(i for i, (c0, cn) in enumerate(dchunks)
                              if c0 <= g < c0 + cn)
                    c0 = dchunks[ci][0]
                    # JIT prefetch: entering chunk ci -> issue chunk ci+2
                    if g == c0 and ci + 2 < len(dchunks):
                        load_d(ci + 2)
                    drows = dtiles[ci][:, (g - c0) * PPIX:(g - c0 + 1) * PPIX
                                       ].rearrange("p (r c) -> p r c", c=PADW)
                    if osb is None:
                        ochunk_start = g
                        osb = opool.tile([Q, max(OCH) * PIX], BF16,
                                         tag="osb")

                    # BDW = mask (.) broadcast(W2 row), tap-independent
                    bdw = bdpool.tile([Q, NL * F], BF16, tag="bdw")
                    nc.vector.tensor_mul(
                        out=bdw.rearrange("p (a b) -> p a b", b=F),
                        in0=w2[:, g, :].unsqueeze(1).broadcast_to([Q, NL, F]),
                        in1=mask_sb)

                    # ALL nine per-tap BD_t = knorm[t] * BDW in one DVE op
                    # (outer product via dual broadcast); the per-op fixed
                    # overhead of 9 small tensor_scalars dominated DVE.
                    bdall = bdpool.tile([Q, KS * KS, NL * F], BF16,
                                        tag="bdall")
                    nc.vector.tensor_mul(
                        out=bdall,
                        in0=bdw.unsqueeze(1).broadcast_to(
                            [Q, KS * KS, NL * F]),
                        in1=knorm[:, g, :].unsqueeze(2).broadcast_to(
                            [Q, KS * KS, NL * F]))

                    pm0 = pmpool.tile([Q, HALF], F32, tag="pm")
                    pm1 = pmpool.tile([Q, HALF], F32, tag="pm")
                    pms = [pm0, pm1]
                    for t in range(KS * KS):
                        ky, kx = divmod(t, KS)
                        for h in range(2):
                            rhs = drows[:, h * 12 + ky:h * 12 + ky + 12,
                                        kx:kx + P]
                            nc.tensor.matmul(
                                pms[h], lhsT=bdall[:, t, :], rhs=rhs,
                                start=(t == 0), stop=(t == KS * KS - 1),
                                skip_group_check=True)

                    gl = g - ochunk_start
                    for h in range(2):
                        nc.scalar.activation(
                            out=osb[:, gl * PIX + h * HALF:
                                    gl * PIX + (h + 1) * HALF],
                            in_=pms[h],
                            func=mybir.ActivationFunctionType.Copy,
                            bias=0.0, scale=r2[:, g:g + 1])

                    for _ in range(per_group):
                        if pending:
                            emit_gap(pending.pop(0))
                    if g - s0 + 1 == wgen_at and si + 1 < len(supers):
                        while pending:
                            emit_gap(pending.pop(0))
                        if si + 1 >= EARLY_A:
                            emit_wgenA(si + 1)
                        emit_wgenB(si + 1)

                    if g + 1 in oends:
                        on = g - ochunk_start + 1
                        nc.sync.dma_start(
                            out=outd[:, ochunk_start:ochunk_start + on
                                     ].rearrange("p g f -> p (g f)"),
                            in_=osb[:, :on * PIX])
                        osb = None

    nc.compile()
    return nc


def _host_prep(guidance, depth, conv_w, conv_b, dense_w, dense_b):
    B, H, W, _ = guidance.shape
    nh, nw = H // P, W // P
    NB = B * nh * nw

    def to_samples(x):
        # (B,H,W,F) -> (NB, P, P, F), sample order = flat (b, i, j)
        return (x.reshape(B, nh, P, nw, P, F)
                 .transpose(0, 1, 3, 2, 4, 5)
                 .reshape(NB, P, P, F))

    gs = to_samples(np.ascontiguousarray(guidance))
    ds = to_samples(np.ascontiguousarray(depth))

    in_maps = []
    for c in range(NCORES):
        gsl = gs[c * 512:(c + 1) * 512]
        dsl = ds[c * 512:(c + 1) * 512]
        gpad = np.zeros((SPC, P, P, F), np.float32)
        gpad[:512] = gsl
        dpad = np.zeros((SPC, PADW, PADW, F), np.float32)
        dpad[:512, 1:P + 1, 1:P + 1] = dsl
        # (SPC, y, x, ch) -> [NGROUP, 126, pix]  with q = n_local*9 + ch
        gq = (gpad.reshape(NGROUP, NL, P, P, F)
                  .transpose(1, 4, 0, 2, 3)
                  .reshape(Q, NGROUP, PIX))
        dq = (dpad.reshape(NGROUP, NL, PADW, PADW, F)
                  .transpose(1, 4, 0, 2, 3)
                  .reshape(Q, NGROUP, PPIX))
        in_maps.append({"gin": np.ascontiguousarray(gq).astype(NPBF),
                        "din": np.ascontiguousarray(dq).astype(NPBF)})

    eye = np.eye(NL, dtype=np.float32)
    lhsA = np.zeros((KS * KS, Q, Q), np.float32)
    for t in range(KS * KS):
        ky, kx = divmod(t, KS)
        lhsA[t] = np.kron(eye, conv_w[ky, kx])
    lhsA = np.ascontiguousarray(lhsA.transpose(1, 0, 2))      # [Q, 9, Q]
    lhsD = np.zeros((F, Q + 1, Q), np.float32)
    lhsD2 = np.zeros((F, Q + 1, Q), np.float32)
    dws = dense_w.astype(np.float32) / PIX  # gap arrives as a SUM over pixels
    for j in range(F):
        lhsD[j, :Q] = np.kron(eye, dws[:, j * F:(j + 1) * F])
        lhsD[j, Q] = np.tile(dense_b[j * F:(j + 1) * F], NL)
        lhsD2[j, :Q] = np.kron(eye, dws[:, j::F])
        lhsD2[j, Q] = np.tile(dense_b[j::F], NL)
    lhsD = np.ascontiguousarray(lhsD.transpose(1, 0, 2))      # [Q+1, 9, Q]
    lhsD2 = np.ascontiguousarray(lhsD2.transpose(1, 0, 2))
    mask = np.kron(eye, np.ones((F, F), np.float32))
    convb = np.tile(conv_b.astype(np.float32), NL)[:, None]

    cpk = np.zeros((Q + 1, 3530), NPBF)
    cpk[0:Q, 0:1134] = lhsA.astype(NPBF).reshape(Q, 1134)
    cpk[:, 1134:2268] = lhsD.astype(NPBF).reshape(Q + 1, 1134)
    cpk[:, 2268:3402] = lhsD2.astype(NPBF).reshape(Q + 1, 1134)
    cpk[0:Q, 3402:3528] = mask.astype(NPBF)
    cpk[0:Q, 3528:3530] = (np.ascontiguousarray(convb)
                           .view(np.uint16).view(NPBF))
    consts = {"cpk": cpk}
    for m in in_maps:
        m.update(consts)
    return in_maps


_CACHED_NC = None


def run(inputs, trace=False, **kw):
    """Build (cached), run on 8 cores, return (full_output, BassKernelResults)."""
    global _CACHED_NC
    inputs = {k: np.asarray(v, np.float32) for k, v in inputs.items()}
    in_maps = _host_prep(**inputs)
    if _CACHED_NC is None:
        _CACHED_NC = build_program()
    res = run_bass_kernel_spmd(_CACHED_NC, in_maps, list(range(NCORES)),
                               trace=trace, **kw)
    outs = []
    for c in range(NCORES):
        o = np.asarray(res.results[c]["out"]).astype(np.float32)
        o = o.reshape(NL, F, NGROUP, P, P)
        o = o.transpose(2, 0, 3, 4, 1).reshape(SPC, P, P, F)[:512]
        outs.append(o)
    full = np.concatenate(outs, 0)  # (4096, 24, 24, 9) in (b, i, j) order
    B, H, W = 16, 384, 384
    return full.reshape(B, H, W, F), res


def kernel(**inputs):
    out, _ = run(inputs, trace=False)
    return out


# revision 79
# speedup vs baseline: 1.0195x; 1.0045x over previous
"""Trainium2 Bass kernel for the Guided-Conv problem (restructured, bf16).

Math (per independent sample n, of NB = 4096):
  g_n, d_n : 24x24x9 patches of guidance / depth.
  c_n      = conv2d(g_n, conv_w, stride 8, VALID-from-SAME) + conv_b -> 3x3x9
  k_n[i]   = c_n[:, :, i] / max(||c_n[:, :, i]||_2, 1)    (per-channel 3x3 taps)
  gap_n    = mean(g_n, (y, x))                            -> 9
  W2_n     = (gap_n @ dense_w + dense_b).reshape(9, 9)    (i2 -> o2)
  r2_n[o]  = 1 / max(||W2_n[:, o]||_2, 1)
  out_n    = (depthwise(d_n, k_n) @ W2_n) * r2_n          -> 24x24x9

Device strategy (per core: 512 samples + 6 pad = 37 groups of 14):
  Partition layout q = n_local*9 + ch on 126 partitions; free = pixels.
  - Everything bf16 on the wire (validated: rel err ~6e-3 < 2e-2 gate);
    PSUM accumulation stays fp32; output returned bf16, host upcasts.
  - Weight gen via block-diagonal matmuls (lhsT = kron(eye14, w) built on
    host). gap sums via ACT copy+accum_out, interleaved into the previous
    supertile's conv loop; the dense/norm chain for supertile s+1 is
    emitted mid-conv(s) so its cross-engine latency hides under matmuls.
  - Main conv: per group, ALL nine tap matrices BD_t = mask (.) W2row (.)
    knorm[t] are built in TWO DVE ops (broadcast outer products); the PE
    accumulates 9 taps x 2 psum halves with self-loaded bf16 weights
    (LDWEIGHTS hides under the 288-col matmul streams at full clock).
  - r2 applied as the per-partition ACT scale on the PSUM->SBUF copy.
  DMA discipline (the hard-won part): engines round-robin over ALL
  outstanding transfers, so inputs are loaded JIT via pool recycling
  (bufs=2) -- an issue waits until the buffer two chunks back is consumed,
  keeping ~2 transfers in flight so completion tracks need. Consts ride
  in one packed buffer split into 8 partition-sliced DMAs (a contiguous
  transfer coalesces onto a single ~25GB/s engine; slices parallelize).
  All loads on the sync queue, stores on sync late; compute queues never
  issue DMAs.
"""

import numpy as np
import ml_dtypes

import concourse.bass as bass
from concourse import bacc
import concourse.mybir as mybir
from concourse.tile import TileContext
from concourse.bass_utils import run_bass_kernel_spmd

F = 9          # channels
P = 24         # patch size
PADW = 26      # padded patch width (SAME conv, pad 1)
KS = 3         # generated kernel size
NCORES = 8
NL = 14        # samples per group
Q = NL * F     # 126 used partitions
NGROUP = 37    # groups per core (36 full + 1 padded)
SPC = NGROUP * NL  # 518 sample slots per core (512 real)
PIX = P * P        # 576
PPIX = PADW * PADW  # 676
HALF = PIX // 2    # 288, pixels per PSUM chunk (<=512 fp32/bank)
SUPER = [4, 8, 12, 13]   # weight-gen supertile sizes (sum = 37)
DCH = [4, 6, 6, 7, 7, 7]  # din chunk sizes (sum = 37)
OCH = [8, 8, 8, 6, 4, 2, 1]  # groups per output-store DMA (small tail)

F32 = mybir.dt.float32
BF16 = mybir.dt.bfloat16
NPBF = ml_dtypes.bfloat16


def build_program():
    nc = bacc.Bacc("TRN2", target_bir_lowering=False, debug=False,
                   num_devices=NCORES)

    gin = nc.dram_tensor("gin", [Q, NGROUP, PIX], BF16, kind="ExternalInput").ap()
    din = nc.dram_tensor("din", [Q, NGROUP, PPIX], BF16, kind="ExternalInput").ap()
    # all consts packed into one buffer -> ONE early DMA:
    # [0:1134) lhsA (126 rows), [1134:2268) lhsD (127), [2268:3402) lhsD2
    # (127), [3402:3528) mask (126), [3528:3530) conv_b as fp32 bytes.
    # Row stride padded to 4096 so the DRAM read is NON-contiguous -- a
    # fully contiguous transfer coalesces onto a single DMA engine.
    cpk = nc.dram_tensor("cpk", [Q + 1, 3530], BF16, kind="ExternalInput").ap()
    outd = nc.dram_tensor("out", [Q, NGROUP, PIX], BF16, kind="ExternalOutput").ap()

    supers = []
    g0 = 0
    for ng in SUPER:
        supers.append((g0, ng))
        g0 += ng
    dchunks = []
    c0 = 0
    for cn in DCH:
        dchunks.append((c0, cn))
        c0 += cn

    with TileContext(nc) as tc:
        with (
            tc.tile_pool(name="consts", bufs=1) as cpool,
            tc.tile_pool(name="gpool", bufs=3) as gpool,
            tc.tile_pool(name="dpool", bufs=2) as dpool,
            tc.tile_pool(name="opool", bufs=3) as opool,
            tc.tile_pool(name="small", bufs=1) as spool,
            tc.tile_pool(name="gapp", bufs=2) as gappool,
            tc.tile_pool(name="bd", bufs=14) as bdpool,
            tc.tile_pool(name="ps_c", bufs=1, space="PSUM") as pcpool,
            tc.tile_pool(name="ps_d", bufs=1, space="PSUM") as pdpool,
            tc.tile_pool(name="ps_main", bufs=5, space="PSUM") as pmpool,
        ):
            # ---- all input DMAs hoisted to program start, on idle queues
            # (issue cost is ~126 descriptors each; keep off the ACT/DVE
            # compute queues and use few, large transfers -- each transfer
            # is spread over all ~14 DMA engines by the packetizer).
            # Strict need-order on ONE ring: transfers drain FIFO per ring
            # round-robined over the shared DMA engines, so bulk loads
            # issued later cannot starve latency-critical small ones.
            # 8 partition-sliced loads: issue order = packet order in the
            # engine FIFOs, and each issue round-robins to a different
            # engine, so slices transfer in parallel ahead of the bulk.
            csb = cpool.tile([Q + 1, 3530], BF16, tag="cpk")
            for p0 in range(0, Q + 1, 16):
                p1 = min(p0 + 16, Q + 1)
                nc.sync.dma_start(out=csb[p0:p1, :], in_=cpk[p0:p1, :])
            lhsA_sb = csb[0:Q, 0:1134].rearrange("p (t q) -> p t q", q=Q)
            lhsD_sb = csb[:, 1134:2268].rearrange("p (t q) -> p t q", q=Q)
            lhsD2_sb = csb[:, 2268:3402].rearrange("p (t q) -> p t q", q=Q)
            mask_sb = csb[0:Q, 3402:3528].rearrange("p (a b) -> p a b", b=F)
            # JIT-staged loading: DMA engines round-robin over ALL
            # outstanding transfers, so hoisting everything means nothing
            # lands until the aggregate drain (~50us). Instead, pool
            # recycling (bufs=2) makes each load's issue WAIT until the
            # buffer two chunks back is consumed -- only ~2 transfers
            # outstanding, completion tracks need.
            GSBW = max(ng for _, ng in supers) * PIX
            DSBW = max(cn for _, cn in dchunks) * PPIX
            gtiles = {}
            dtiles = {}

            def load_g(si):
                s0, ng = supers[si]
                t = gpool.tile([Q, GSBW], BF16, tag="gsb")
                gtiles[si] = t
                nc.sync.dma_start(
                    out=t[:, :ng * PIX],
                    in_=gin[:, s0:s0 + ng].rearrange("p g f -> p (g f)"))

            def load_d(ci):
                c0, cn = dchunks[ci]
                t = dpool.tile([Q, DSBW], BF16, tag="dsb")
                dtiles[ci] = t
                nc.sync.dma_start(
                    out=t[:, :cn * PPIX],
                    in_=din[:, c0:c0 + cn].rearrange("p g f -> p (g f)"))

            convb_sb = csb[0:Q, 3528:3530].bitcast(F32)
            load_g(0)
            load_d(0)
            load_g(1)
            load_d(1)

            # ---- persistent per-core small tensors ----
            craw = spool.tile([Q, NGROUP, F], F32, tag="craw")     # c + conv_b
            knorm = spool.tile([Q, NGROUP, F], F32, tag="knorm")   # normalized taps
            w2 = spool.tile([Q, NGROUP, F], BF16, tag="w2")        # raw W2 (D2 layout)
            r2 = spool.tile([Q, NGROUP], F32, tag="r2")            # 1/max(n2,1)
            r1 = spool.tile([Q, NGROUP], F32, tag="r1")            # 1/max(n1,1)
            sq = spool.tile([Q, NGROUP, F], F32, tag="sq")         # scratch squares
            s1 = spool.tile([Q, NGROUP], F32, tag="s1")            # scratch sums
            gap32 = spool.tile([Q, NGROUP], F32, tag="gap32")      # fp32 pixel sums

            def emit_gap(g):
                # per-group pixel SUM, alternating between the ACT queue
                # (copy + accum_out) and DVE (tensor_reduce) to balance
                # engine load; emitted interleaved with the previous
                # supertile's conv to avoid serial bursts.
                si = next(i for i, (s0, ng) in enumerate(supers)
                          if s0 <= g < s0 + ng)
                s0 = supers[si][0]
                gv = gtiles[si][:, (g - s0) * PIX:(g - s0 + 1) * PIX]
                if si == 0 and g % 2 == 0:
                    # prologue: split s0's sums across DVE and ACT so the
                    # two chains run in parallel (both engines are idle,
                    # and the first dense waits on ALL of them).
                    nc.vector.tensor_reduce(
                        out=gap32[:, g:g + 1],
                        in_=gv.rearrange("p (o f) -> p o f", o=1),
                        axis=mybir.AxisListType.X, op=mybir.AluOpType.add)
                else:
                    scr = gappool.tile([Q, PIX], BF16, tag="scr")
                    nc.scalar.activation(
                        out=scr, in_=gv,
                        func=mybir.ActivationFunctionType.Copy,
                        bias=0.0, scale=1.0,
                        accum_out=gap32[:, g:g + 1])

            def emit_wgenA(si):
                # conv A (strided conv -> craw): only needs lhsA + gsb, so
                # it can run early and keep the PE warm during the prologue.
                s0, ng = supers[si]
                gsl = slice(s0, s0 + ng)
                gsb = gtiles[si][:, :ng * PIX]
                psc = pcpool.tile([Q, ng, F], F32, tag="psc")
                gwin = gsb.rearrange(
                    "p (g oy yr ox xr) -> p g oy ox yr xr",
                    g=ng, oy=KS, yr=8, ox=KS, xr=8)
                for t in range(KS * KS):
                    ky, kx = divmod(t, KS)
                    nc.tensor.matmul(
                        psc,
                        lhsT=lhsA_sb[:, t, :],
                        rhs=gwin[:, :, :, :, ky, kx],
                        start=(t == 0), stop=(t == KS * KS - 1),
                        skip_group_check=True)

                # craw = psc + conv_b (per-partition bias)
                nc.scalar.activation(
                    out=craw[:, gsl, :], in_=psc,
                    func=mybir.ActivationFunctionType.Identity,
                    bias=convb_sb, scale=1.0)

            def emit_wgenB(si):
                s0, ng = supers[si]
                gsl = slice(s0, s0 + ng)
                # bf16 gap copy for the dense rhs; row 126 reads 1.0 so the
                # K=127 matmul adds dense_b.
                gap = gappool.tile([128, ng], BF16, tag="gap")
                nc.vector.memset(gap, 1.0)
                nc.scalar.copy(out=gap[0:Q, :], in_=gap32[:, gsl])

                # dense layer, both layouts (D for the norm, D2 for values)
                psD = pdpool.tile([Q, F, ng], F32, tag="psD")
                psD2 = pdpool.tile([Q, F, ng], F32, tag="psD2")
                for j in range(F):
                    nc.tensor.matmul(psD[:, j, :], lhsT=lhsD_sb[:, j, :],
                                     rhs=gap[0:Q + 1, :],
                                     start=True, stop=True,
                                     skip_group_check=True)
                for j in range(F):
                    nc.tensor.matmul(psD2[:, j, :], lhsT=lhsD2_sb[:, j, :],
                                     rhs=gap[0:Q + 1, :],
                                     start=True, stop=True,
                                     skip_group_check=True)

                # r2 = 1/max(||W2[:,o]||, 1):  sum_i2 D^2 per (n,o2)
                nc.scalar.square(out=sq[:, gsl, :],
                                 in_=psD.rearrange("p i g -> p g i"))
                nc.vector.tensor_reduce(
                    out=s1[:, gsl], in_=sq[:, gsl, :],
                    axis=mybir.AxisListType.X, op=mybir.AluOpType.add)
                nc.scalar.sqrt(out=s1[:, gsl], in_=s1[:, gsl])
                nc.vector.tensor_scalar_max(r2[:, gsl], s1[:, gsl], 1.0)
                nc.vector.reciprocal(r2[:, gsl], r2[:, gsl])

                # W2 raw values, group-major, stored bf16 for the BDW build
                nc.scalar.copy(out=w2[:, gsl, :],
                               in_=psD2.rearrange("p o g -> p g o"))

                # r1 = 1/max(||c||, 1) per (n, ch); knorm = craw * r1
                nc.scalar.square(out=sq[:, gsl, :], in_=craw[:, gsl, :])
                nc.vector.tensor_reduce(
                    out=s1[:, gsl], in_=sq[:, gsl, :],
                    axis=mybir.AxisListType.X, op=mybir.AluOpType.add)
                nc.scalar.sqrt(out=s1[:, gsl], in_=s1[:, gsl])
                nc.vector.tensor_scalar_max(r1[:, gsl], s1[:, gsl], 1.0)
                nc.vector.reciprocal(r1[:, gsl], r1[:, gsl])
                nc.vector.tensor_mul(
                    out=knorm[:, gsl, :], in0=craw[:, gsl, :],
                    in1=r1[:, gsl].unsqueeze(2).broadcast_to([Q, ng, F]))

            # prologue: conv-A for the first three supertiles keeps the PE
            # busy (and ramping to full clock) while the s0 dense/norm
            # chain's cross-engine latency plays out.
            EARLY_A = min(2, len(supers))
            emit_wgenA(0)
            for g in range(supers[0][0], supers[0][0] + supers[0][1]):
                emit_gap(g)
            for si in range(1, EARLY_A):
                emit_wgenA(si)
            emit_wgenB(0)

            oends = set()
            acc = 0
            for on_ in OCH:
                acc += on_
                oends.add(acc)
            osb = None
            ochunk_start = 0
            for si, (s0, ng) in enumerate(supers):
                # interleave: next supertile's gap sums into this conv loop,
                # and emit its whole weight-gen mid-conv so the dense/norm
                # chain latency hides under this supertile's matmuls.
                if si + 1 < len(supers):
                    nxt0, nxtn = supers[si + 1]
                    pending = list(range(nxt0, nxt0 + nxtn))
                else:
                    pending = []
                wgen_at = max(ng * 3 // 5, 1) if pending else ng + 1
                per_group = -(-len(pending) // wgen_at) if pending else 0
                if si + 1 < len(supers) and si + 1 not in gtiles:
                    load_g(si + 1)

                for g in range(s0, s0 + ng):
                    ci = next(i for i, (c0, cn) in enumerate(dchunks)
                              if c0 <= g < c0 + cn)
                    c0 = dchunks[ci][0]
                    # JIT prefetch: entering chunk ci -> issue chunk ci+2
                    if g == c0 and ci + 2 < len(dchunks):
                        load_d(ci + 2)
                    drows = dtiles[ci][:, (g - c0) * PPIX:(g - c0 + 1) * PPIX
                                       ].rearrange("p (r c) -> p r c", c=PADW)
                    if osb is None:
                        ochunk_start = g
                        osb = opool.tile([Q, max(OCH) * PIX], BF16,
                                         tag="osb")

                    # BDW = mask (.) broadcast(W2 row), tap-independent
                    bdw = bdpool.tile([Q, NL * F], BF16, tag="bdw")
                    nc.vector.tensor_mul(
                        out=bdw.rearrange("p (a b) -> p a b", b=F),
                        in0=w2[:, g, :].unsqueeze(1).broadcast_to([Q, NL, F]),
                        in1=mask_sb)

                    # ALL nine per-tap BD_t = knorm[t] * BDW in one DVE op
                    # (outer product via dual broadcast); the per-op fixed
                    # overhead of 9 small tensor_scalars dominated DVE.
                    bdall = bdpool.tile([Q, KS * KS, NL * F], BF16,
                                        tag="bdall")
                    nc.vector.tensor_mul(
                        out=bdall,
                        in0=bdw.unsqueeze(1).broadcast_to(
                            [Q, KS * KS, NL * F]),
                        in1=knorm[:, g, :].unsqueeze(2).broadcast_to(
                            [Q, KS * KS, NL * F]))

                    pm0 = pmpool.tile([Q, HALF], F32, tag="pm")
                    pm1 = pmpool.tile([Q, HALF], F32, tag="pm")
                    pms = [pm0, pm1]
                    for t in range(KS * KS):
                        ky, kx = divmod(t, KS)
                        for h in range(2):
                            rhs = drows[:, h * 12 + ky:h * 12 + ky + 12,
                                        kx:kx + P]
                            nc.tensor.matmul(
                                pms[h], lhsT=bdall[:, t, :], rhs=rhs,
                                start=(t == 0), stop=(t == KS * KS - 1),
                                skip_group_check=True)

                    gl = g - ochunk_start
                    for h in range(2):
                        nc.scalar.activation(
                            out=osb[:, gl * PIX + h * HALF:
                                    gl * PIX + (h + 1) * HALF],
                            in_=pms[h],
                            func=mybir.ActivationFunctionType.Copy,
                            bias=0.0, scale=r2[:, g:g + 1])

                    for _ in range(per_group):
                        if pending:
                            emit_gap(pending.pop(0))
                    if g - s0 + 1 == wgen_at and si + 1 < len(supers):
                        while pending:
                            emit_gap(pending.pop(0))
                        if si + 1 >= EARLY_A:
                            emit_wgenA(si + 1)
                        emit_wgenB(si + 1)

                    if g + 1 in oends:
                        on = g - ochunk_start + 1
                        nc.sync.dma_start(
                            out=outd[:, ochunk_start:ochunk_start + on
                                     ].rearrange("p g f -> p (g f)"),
                            in_=osb[:, :on * PIX])
                        osb = None

    nc.compile()
    return nc


def _host_prep(guidance, depth, conv_w, conv_b, dense_w, dense_b):
    B, H, W, _ = guidance.shape
    nh, nw = H // P, W // P
    NB = B * nh * nw

    def to_samples(x):
        # (B,H,W,F) -> (NB, P, P, F), sample order = flat (b, i, j)
        return (x.reshape(B, nh, P, nw, P, F)
                 .transpose(0, 1, 3, 2, 4, 5)
                 .reshape(NB, P, P, F))

    gs = to_samples(np.ascontiguousarray(guidance))
    ds = to_samples(np.ascontiguousarray(depth))

    in_maps = []
    for c in range(NCORES):
        gsl = gs[c * 512:(c + 1) * 512]
        dsl = ds[c * 512:(c + 1) * 512]
        gpad = np.zeros((SPC, P, P, F), np.float32)
        gpad[:512] = gsl
        dpad = np.zeros((SPC, PADW, PADW, F), np.float32)
        dpad[:512, 1:P + 1, 1:P + 1] = dsl
        # (SPC, y, x, ch) -> [NGROUP, 126, pix]  with q = n_local*9 + ch
        gq = (gpad.reshape(NGROUP, NL, P, P, F)
                  .transpose(1, 4, 0, 2, 3)
                  .reshape(Q, NGROUP, PIX))
        dq = (dpad.reshape(NGROUP, NL, PADW, PADW, F)
                  .transpose(1, 4, 0, 2, 3)
                  .reshape(Q, NGROUP, PPIX))
        in_maps.append({"gin": np.ascontiguousarray(gq).astype(NPBF),
                        "din": np.ascontiguousarray(dq).astype(NPBF)})

    eye = np.eye(NL, dtype=np.float32)
    lhsA = np.zeros((KS * KS, Q, Q), np.float32)
    for t in range(KS * KS):
        ky, kx = divmod(t, KS)
        lhsA[t] = np.kron(eye, conv_w[ky, kx])
    lhsA = np.ascontiguousarray(lhsA.transpose(1, 0, 2))      # [Q, 9, Q]
    lhsD = np.zeros((F, Q + 1, Q), np.float32)
    lhsD2 = np.zeros((F, Q + 1, Q), np.float32)
    dws = dense_w.astype(np.float32) / PIX  # gap arrives as a SUM over pixels
    for j in range(F):
        lhsD[j, :Q] = np.kron(eye, dws[:, j * F:(j + 1) * F])
        lhsD[j, Q] = np.tile(dense_b[j * F:(j + 1) * F], NL)
        lhsD2[j, :Q] = np.kron(eye, dws[:, j::F])
        lhsD2[j, Q] = np.tile(dense_b[j::F], NL)
    lhsD = np.ascontiguousarray(lhsD.transpose(1, 0, 2))      # [Q+1, 9, Q]
    lhsD2 = np.ascontiguousarray(lhsD2.transpose(1, 0, 2))
    mask = np.kron(eye, np.ones((F, F), np.float32))
    convb = np.tile(conv_b.astype(np.float32), NL)[:, None]

    cpk = np.zeros((Q + 1, 3530), NPBF)
    cpk[0:Q, 0:1134] = lhsA.astype(NPBF).reshape(Q, 1134)
    cpk[:, 1134:2268] = lhsD.astype(NPBF).reshape(Q + 1, 1134)
    cpk[:, 2268:3402] = lhsD2.astype(NPBF).reshape(Q + 1, 1134)
    cpk[0:Q, 3402:3528] = mask.astype(NPBF)
    cpk[0:Q, 3528:3530] = (np.ascontiguousarray(convb)
                           .view(np.uint16).view(NPBF))
    consts = {"cpk": cpk}
    for m in in_maps:
        m.update(consts)
    return in_maps


_CACHED_NC = None


def run(inputs, trace=False, **kw):
    """Build (cached), run on 8 cores, return (full_output, BassKernelResults)."""
    global _CACHED_NC
    inputs = {k: np.asarray(v, np.float32) for k, v in inputs.items()}
    in_maps = _host_prep(**inputs)
    if _CACHED_NC is None:
        _CACHED_NC = build_program()
    res = run_bass_kernel_spmd(_CACHED_NC, in_maps, list(range(NCORES)),
                               trace=trace, **kw)
    outs = []
    for c in range(NCORES):
        o = np.asarray(res.results[c]["out"]).astype(np.float32)
        o = o.reshape(NL, F, NGROUP, P, P)
        o = o.transpose(2, 0, 3, 4, 1).reshape(SPC, P, P, F)[:512]
        outs.append(o)
    full = np.concatenate(outs, 0)  # (4096, 24, 24, 9) in (b, i, j) order
    B, H, W = 16, 384, 384
    return full.reshape(B, H, W, F), res


def kernel(**inputs):
    out, _ = run(inputs, trace=False)
    return out
